# revision 9
# baseline (speedup 1.0000x reference)
"""DistMult scoring kernel for Trainium2 (8 NeuronCores, SPMD).

score = sigmoid( (ent_emb[h] * diag(rel_emb[r])) @ ent_emb[t].T )
  batch_h/t/r: (2048,) int; ent_emb: (400000, 256) f32;
  rel_emb: (500, 256, 256) diagonal -> only its (500, 256) diagonal matters.

Strategy (all bf16 on device, fp32 accumulate in PSUM):
  - Heads sharded by GLOBAL SORTED order: core c computes score rows for the
    256 heads at sorted positions [256c, 256c+256). Tails are shared: every
    core uses all 2048 tails in GLOBAL SORTED order. The host un-permutes
    rows/columns when assembling the full (2048, 2048) output.
  - Tails: 16 dma_gather(transpose=True) window calls (128 sorted indices per
    call; each window spans <= 32767 rows so int16 offsets reach it). Window
    base addresses are compile-time AP offsets derived from the call's data
    (the kernel is JIT-specialized per window-base tuple and cached).
    dma_gather issues in ~90ns on gpsimd and runs async on the 4 SWDGE
    queues, landing e-on-partition tiles directly - no PE transposes.
  - Heads: 2 x 128-row indirect DMA (int32 reach beyond the int16 window).
  - Rels: no gather at all - a one-hot selection matmul on the (idle, warming)
    PE picks each head's relation diagonal out of the 500-row table, row-major;
    DVE folds it into the head rows; 4 PE transposes produce hrT.
  - Score matmuls on PE (bf16, k=256 over 2 PSUM-accumulated 128-tiles);
    sigmoid on Scalar straight out of PSUM with bf16 output; sync HWDGE
    writes the (256, 2048) bf16 block.
  - SWDGE completion sems rotate over 8 lanes; a post-schedule pass rewrites
    each gather's queue_num to a pure function of its lane so each sem is
    driven by exactly one queue (the generic indirect DMAs are queue-0-only).
  - Dummy matmuls on a zero tile + the one-hot matmuls keep the PE HAM clock
    ramped before the real matmuls; the ~9us one-time Q7 'mlp' library load
    (required by dma_gather) is issued as early as possible.
"""

import sys

if "/opt/trn_rl_repo" not in sys.path:
    sys.path.insert(0, "/opt/trn_rl_repo")

import numpy as np
import ml_dtypes

import concourse.bass as bass
import concourse.tile as tile
from concourse import bacc, mybir


def _mlp_lib():
    from concourse.library_config import mlp

    return mlp


B = 2048
E = 256
N_ENT = 400000
N_REL = 500
CORES = 8
M = B // CORES  # 256 head rows per core
P = 128
WIN = 32768  # dma_gather int16 reach in table rows

F32 = mybir.dt.float32
BF16 = mybir.dt.bfloat16
I32 = mybir.dt.int32
I16 = mybir.dt.int16

NWARM_EARLY = 8  # PE warmups before the one-hot matmuls
NWARM_BRIDGE = 28  # PE warmups bridging one-hot matmuls -> real matmuls


def build_nc(G, tail_bases):
    """G = number of 128-wide tail window buckets; tail_bases = their row bases."""
    nc = bacc.Bacc(
        "TRN2", target_bir_lowering=False, debug=False,
        num_devices=CORES, num_swdge_queues=4,
    )

    entb = nc.dram_tensor("entb", [N_ENT, E], BF16, kind="ExternalInput").ap()
    relb = nc.dram_tensor("relb", [512, E], BF16, kind="ExternalInput").ap()
    tidx = nc.dram_tensor("tidx", [P, 8 * G], I16, kind="ExternalInput").ap()
    hidx = nc.dram_tensor("hidx", [P, 2], I32, kind="ExternalInput").ap()
    rvals = nc.dram_tensor("rvals", [P, M], F32, kind="ExternalInput").ap()
    riota = nc.dram_tensor("riota", [P, 4], F32, kind="ExternalInput").ap()
    ident = nc.dram_tensor("ident", [P, P], BF16, kind="ExternalInput").ap()
    score = nc.dram_tensor("score", [M, P * G], BF16, kind="ExternalOutput").ap()

    with tile.TileContext(nc) as tc:
        with (
            tc.tile_pool(name="idxp", bufs=1) as idx_pool,
            tc.tile_pool(name="gat", bufs=1) as gat_pool,
            tc.tile_pool(name="outp", bufs=8) as out_pool,
            tc.tile_pool(name="psmm", bufs=4, space="PSUM") as psum_mm,
            tc.tile_pool(name="psr", bufs=2, space="PSUM") as psum_r,
            tc.tile_pool(name="pst", bufs=1, space="PSUM") as psum_t,
            tc.tile_pool(name="pswm", bufs=1, space="PSUM") as psum_wm,
        ):
            # --- early: library load, zero tile, small input DMAs ---
            nc.gpsimd.load_library(_mlp_lib())
            wmt = gat_pool.tile([P, P], BF16, tag="wmt")
            nc.vector.memset(wmt[:], 0.0)

            hidx_sb = idx_pool.tile([P, 2], I32)
            nc.sync.dma_start(hidx_sb[:], hidx[:])
            tidx_sb = idx_pool.tile([P, 8 * G], I16)
            nc.sync.dma_start(tidx_sb[:], tidx[:])
            rvals_sb = idx_pool.tile([P, M], F32)
            nc.scalar.dma_start(rvals_sb[:], rvals[:])
            riota_sb = idx_pool.tile([P, 4], F32)
            nc.scalar.dma_start(riota_sb[:], riota[:])
            ident_sb = idx_pool.tile([P, P], BF16)
            nc.scalar.dma_start(ident_sb[:], ident[:])
            relb_sb = gat_pool.tile([P, 4, E], BF16, tag="relsb")
            for t in range(4):
                nc.sync.dma_start(relb_sb[:, t, :], relb[t * P : (t + 1) * P, :])

            # --- Pool-engine DMAs (queues rewritten post-schedule):
            #     heads indirect x2, then the 16 tail window gathers ---
            hrows = gat_pool.tile([P, 2, E], BF16, tag="hrows")
            for j in range(2):
                nc.gpsimd.indirect_dma_start(
                    out=hrows[:, j, :],
                    out_offset=None,
                    in_=entb[:],
                    in_offset=bass.IndirectOffsetOnAxis(
                        ap=hidx_sb[:, j : j + 1], axis=0
                    ),
                )
            tbig = gat_pool.tile([P, G, 2, P], BF16, tag="tbig")
            for w in range(G):
                base = tail_bases[w]
                nc.gpsimd.dma_gather(
                    out_ap=tbig[:, w, :, :],
                    in_ap=entb[base : base + WIN, :],
                    idxs_ap=tidx_sb[:, 8 * w : 8 * w + 8],
                    num_idxs=P, num_idxs_reg=P, elem_size=E,
                    transpose=True, queue_num=0,
                )

            # --- one-hot rel selection, fold into heads, transpose ---
            # OH[r_part, t, m] = (batch_r[m] == 128t + r_part)
            oh = gat_pool.tile([P, 4, M], BF16, tag="oh")
            for t in range(4):
                nc.vector.tensor_scalar(
                    oh[:, t, :], rvals_sb[:], riota_sb[:, t : t + 1], None,
                    op0=mybir.AluOpType.is_equal,
                )

            # PE warmups (HAM clock ramp) before the one-hot matmuls
            wpsum = psum_wm.tile([P, P], F32)
            for _ in range(NWARM_EARLY):
                nc.tensor.matmul(wpsum[:], lhsT=wmt[:], rhs=wmt[:], start=True, stop=True)

            # rel_sel[m_tile i] = sum_t OH[:, t, mi]^T @ relb[t]  -> [128m, 256e]
            psr_tiles = []
            for i in range(2):
                psr = psum_r.tile([P, E], F32, tag="psr", name=f"psr{i}")
                for t in range(4):
                    nc.tensor.matmul(
                        psr[:],
                        lhsT=oh[:, t, i * P : (i + 1) * P],
                        rhs=relb_sb[:, t, :],
                        start=(t == 0),
                        stop=(t == 3),
                    )
                psr_tiles.append(psr)

            # bridge warmups while gathers land
            for _ in range(NWARM_BRIDGE):
                nc.tensor.matmul(wpsum[:], lhsT=wmt[:], rhs=wmt[:], start=True, stop=True)

            # hr rows = heads * rel_sel (DVE, psum read), then PE transposes
            hr_rows = gat_pool.tile([P, 2, E], BF16, tag="hrr")
            for i in range(2):
                nc.vector.tensor_mul(hr_rows[:, i, :], hrows[:, i, :], psr_tiles[i][:])
            hrT = gat_pool.tile([P, 2, M], BF16, tag="hrT")
            for k in range(2):
                pst = psum_t.tile([P, M], BF16, tag="pst", name=f"pst{k}")
                for i in range(2):
                    nc.tensor.transpose(
                        pst[:, i * P : (i + 1) * P],
                        hr_rows[:, i, k * P : (k + 1) * P],
                        ident_sb[:],
                    )
                nc.vector.tensor_copy(hrT[:, k, :], pst[:])

            # --- score matmuls + sigmoid + out, n-chunks of 512 cols ---
            n_chunks = (G + 3) // 4
            for c in range(n_chunks):
                w0 = 4 * c
                wn = min(4, G - w0)
                ncols = wn * P
                for i in range(M // P):
                    ps = psum_mm.tile([P, ncols], F32, tag="ps", name=f"ps{c}_{i}")
                    for k in range(2):
                        nc.tensor.matmul(
                            ps[:],
                            lhsT=hrT[:, k, i * P : (i + 1) * P],
                            rhs=tbig[:, w0 : w0 + wn, k, :],
                            start=(k == 0),
                            stop=(k == 1),
                        )
                    ob = out_pool.tile([P, ncols], BF16, tag="ob", name=f"ob{c}_{i}")
                    nc.scalar.activation(
                        ob[:], ps[:], mybir.ActivationFunctionType.Sigmoid
                    )
                    nc.sync.dma_start(
                        score[i * P : (i + 1) * P, w0 * P : w0 * P + ncols], ob[:]
                    )

    # Tile assigns DMASW completion sems round-robin over 8 lanes in
    # scheduled order, and SWDGE shadow-sem bookkeeping requires each sem to
    # be driven by exactly one queue. Rewrite each gather's queue to a pure
    # function of its lane; lanes used by the queue-0-only indirect DMAs
    # stay on queue 0, the rest spread evenly over queues 1..3.
    import re as _re

    pool_dmas = []
    for bb in nc.main_func.blocks:
        for inst in bb.instructions:
            if inst.engine != mybir.EngineType.Pool:
                continue
            si = inst.sync_info
            if not si or not si.on_update:
                continue
            m = _re.match(r"DMASW(\d+)_", si.on_update[0].ant_name or "")
            if not m:
                continue
            pool_dmas.append((inst, int(m.group(1))))
    indirect_lanes = {
        lane for inst, lane in pool_dmas if isinstance(inst, mybir.InstDMACopy)
    }
    qmap = {lane: 0 for lane in indirect_lanes}
    free_lanes = [ln for ln in range(8) if ln not in indirect_lanes]
    fill = ([1, 2, 3] if indirect_lanes else [0, 1, 2, 3]) * 8
    for i, ln in enumerate(free_lanes):
        qmap[ln] = fill[i]
    for inst, lane in pool_dmas:
        if isinstance(inst, mybir.InstDMAGatherAnt):
            inst.queue_num = qmap[lane]

    nc.compile()
    return nc


_NC_CACHE = {}


def _get_nc(G, tail_bases):
    key = (G, tuple(tail_bases))
    if key not in _NC_CACHE:
        _NC_CACHE[key] = build_nc(G, tail_bases)
    return _NC_CACHE[key]


def _wrap16(idx, reps=8):
    """Position i of a gather call reads idxs[i % 16, i // 16]; replicate to 128 rows."""
    n = idx.shape[0]
    w = idx.reshape(n // 16, 16).T
    return np.ascontiguousarray(np.tile(w, (reps, 1)))


def _plan_tail_buckets(bt_sorted):
    """Greedy exact-128 buckets of sorted tail indices; pad a bucket (repeating
    its first index) when 128 consecutive sorted values span > 32767 rows.
    Returns (bases, lo_idx [G*128] int16, keep [G*128] bool)."""
    n = bt_sorted.shape[0]
    bases, lo_all, keep_all = [], [], []
    pos = 0
    while pos < n:
        chunk = bt_sorted[pos : pos + P]
        span = int(chunk[-1]) - int(chunk[0])
        if span <= WIN - 1:
            take = len(chunk)
        else:
            take = int(np.searchsorted(chunk, chunk[0] + WIN, side="left"))
        vals = chunk[:take]
        pad = P - take
        if pad:
            vals = np.concatenate([vals, np.full(pad, vals[0], dtype=vals.dtype)])
        base = min(int(vals.min()), N_ENT - WIN)
        bases.append(base)
        lo_all.append((vals - base).astype(np.int16))
        keep_all.append(np.arange(P) < take)
        pos += take
    return bases, np.concatenate(lo_all), np.concatenate(keep_all)


def prepare(batch_h, batch_t, batch_r, ent_emb, rel_emb):
    bh = np.asarray(batch_h).astype(np.int64)
    bt = np.asarray(batch_t).astype(np.int64)
    br = np.asarray(batch_r).astype(np.int64)

    entb = np.asarray(ent_emb).astype(ml_dtypes.bfloat16)
    rel_np = np.asarray(rel_emb)
    rel_diag = rel_np[:, np.arange(E), np.arange(E)].astype(ml_dtypes.bfloat16)
    relb = np.zeros((512, E), dtype=ml_dtypes.bfloat16)
    relb[:N_REL] = rel_diag

    # tails: global sort -> window buckets (shared by all cores)
    t_order = np.argsort(bt, kind="stable")
    bases, t_lo, t_keep = _plan_tail_buckets(bt[t_order])
    G = len(bases)
    tidx = _wrap16(t_lo)  # [128, 8G]

    # heads: global sort -> per-core slices of 256
    h_order = np.argsort(bh, kind="stable")
    riota = np.ascontiguousarray(
        np.arange(P, dtype=np.float32)[:, None]
        + 128.0 * np.arange(4, dtype=np.float32)[None, :]
    )
    identity = np.eye(P, dtype=ml_dtypes.bfloat16)

    in_maps = []
    for c in range(CORES):
        rows = h_order[c * M : (c + 1) * M]  # original batch positions
        hvals = bh[rows].astype(np.int32)
        hidx = np.ascontiguousarray(hvals.reshape(2, P).T)  # [128, 2] col j = rows 128j+p
        rv = br[rows].astype(np.float32)
        rvals = np.ascontiguousarray(np.tile(rv[None, :], (P, 1)))
        in_maps.append(
            {
                "entb": entb, "relb": relb, "tidx": tidx, "hidx": hidx,
                "rvals": rvals, "riota": riota, "ident": identity,
            }
        )
    meta = {
        "G": G, "bases": tuple(int(b) for b in bases),
        "t_order": t_order, "t_keep": t_keep, "h_order": h_order,
    }
    return in_maps, meta


def run(batch_h, batch_t, batch_r, ent_emb, rel_emb, trace=False, tmpdir=None):
    from concourse.bass_utils import run_bass_kernel_spmd

    in_maps, meta = prepare(batch_h, batch_t, batch_r, ent_emb, rel_emb)
    nc = _get_nc(meta["G"], meta["bases"])
    kwargs = {}
    if trace:
        kwargs = {"trace": True, "tmpdir": tmpdir}
    res = run_bass_kernel_spmd(nc, in_maps, core_ids=list(range(CORES)), **kwargs)

    keep = meta["t_keep"]
    col_src = np.nonzero(keep)[0]  # device cols holding real sorted positions
    t_cols = meta["t_order"]  # sorted position -> original batch column
    full = np.empty((B, B), dtype=np.float32)
    for c in range(CORES):
        blk = np.asarray(res.results[c]["score"])  # [256, 128G] bf16
        rows = meta["h_order"][c * M : (c + 1) * M]
        full[np.ix_(rows, t_cols)] = blk[:, col_src].astype(np.float32)
    return full, res


def kernel(batch_h, batch_t, batch_r, ent_emb, rel_emb):
    score, _ = run(batch_h, batch_t, batch_r, ent_emb, rel_emb)
    return score


# revision 11
# speedup vs baseline: 1.0733x; 1.0733x over previous
"""DistMult scoring kernel for Trainium2 (8 NeuronCores, SPMD).

score = sigmoid( (ent_emb[h] * diag(rel_emb[r])) @ ent_emb[t].T )
  batch_h/t/r: (2048,) int; ent_emb: (400000, 256) f32;
  rel_emb: (500, 256, 256) diagonal -> only its (500, 256) diagonal matters.

Strategy (all bf16 on device, fp32 accumulate in PSUM):
  - Heads sharded by GLOBAL SORTED order: core c computes score rows for the
    256 heads at sorted positions [256c, 256c+256). Tails are shared: every
    core uses all 2048 tails in GLOBAL SORTED order. The host un-permutes
    rows/columns when assembling the full (2048, 2048) output.
  - Tails: 16 dma_gather(transpose=True) window calls (128 sorted indices per
    call; each window spans <= 32767 rows so int16 offsets reach it). Window
    base addresses are compile-time AP offsets derived from the call's data
    (the kernel is JIT-specialized per window-base tuple and cached).
    dma_gather issues in ~90ns on gpsimd and runs async on the 4 SWDGE
    queues, landing e-on-partition tiles directly - no PE transposes.
  - Heads: 2 x 128-row indirect DMA (int32 reach beyond the int16 window).
  - Rels: no gather at all - a one-hot selection matmul on the (idle, warming)
    PE picks each head's relation diagonal out of the 500-row table, row-major;
    DVE folds it into the head rows; 4 PE transposes produce hrT.
  - Score matmuls on PE (bf16, k=256 over 2 PSUM-accumulated 128-tiles);
    sigmoid on Scalar straight out of PSUM with bf16 output; sync HWDGE
    writes the (256, 2048) bf16 block.
  - SWDGE completion sems rotate over 8 lanes; a post-schedule pass rewrites
    each gather's queue_num to a pure function of its lane so each sem is
    driven by exactly one queue (the generic indirect DMAs are queue-0-only).
  - Dummy matmuls on a zero tile + the one-hot matmuls keep the PE HAM clock
    ramped before the real matmuls; the ~9us one-time Q7 'mlp' library load
    (required by dma_gather) is issued as early as possible.
"""

import sys

if "/opt/trn_rl_repo" not in sys.path:
    sys.path.insert(0, "/opt/trn_rl_repo")

import numpy as np
import ml_dtypes

import concourse.bass as bass
import concourse.tile as tile
from concourse import bacc, mybir


def _mlp_lib():
    from concourse.library_config import mlp

    return mlp


B = 2048
E = 256
N_ENT = 400000
N_REL = 500
CORES = 8
M = B // CORES  # 256 head rows per core
P = 128
WIN = 32768  # dma_gather int16 reach in table rows

F32 = mybir.dt.float32
BF16 = mybir.dt.bfloat16
I32 = mybir.dt.int32
I16 = mybir.dt.int16

NWARM_EARLY = 8  # PE warmups before the one-hot matmuls
NWARM_BRIDGE = 28  # PE warmups bridging one-hot matmuls -> real matmuls


def build_nc(G, tail_bases):
    """G = number of 128-wide tail window buckets; tail_bases = their row bases."""
    nc = bacc.Bacc(
        "TRN2", target_bir_lowering=False, debug=False,
        num_devices=CORES, num_swdge_queues=4,
    )

    entb = nc.dram_tensor("entb", [N_ENT, E], BF16, kind="ExternalInput").ap()
    relb = nc.dram_tensor("relb", [512, E], BF16, kind="ExternalInput").ap()
    tidx = nc.dram_tensor("tidx", [P, 8 * G], I16, kind="ExternalInput").ap()
    hidx = nc.dram_tensor("hidx", [P, 2], I32, kind="ExternalInput").ap()
    rvals = nc.dram_tensor("rvals", [P, M], F32, kind="ExternalInput").ap()
    riota = nc.dram_tensor("riota", [P, 4], F32, kind="ExternalInput").ap()
    ident = nc.dram_tensor("ident", [P, P], BF16, kind="ExternalInput").ap()
    score = nc.dram_tensor("score", [M, P * G], BF16, kind="ExternalOutput").ap()

    with tile.TileContext(nc) as tc:
        with (
            tc.tile_pool(name="idxp", bufs=1) as idx_pool,
            tc.tile_pool(name="gat", bufs=1) as gat_pool,
            tc.tile_pool(name="outp", bufs=8) as out_pool,
            tc.tile_pool(name="psmm", bufs=3, space="PSUM") as psum_mm,
            tc.tile_pool(name="psr", bufs=2, space="PSUM") as psum_r,
            tc.tile_pool(name="pst", bufs=2, space="PSUM") as psum_t,
            tc.tile_pool(name="pswm", bufs=1, space="PSUM") as psum_wm,
        ):
            # --- early: library load, zero tile, small input DMAs ---
            nc.gpsimd.load_library(_mlp_lib())
            wmt = gat_pool.tile([P, P], BF16, tag="wmt")
            nc.vector.memset(wmt[:], 0.0)

            hidx_sb = idx_pool.tile([P, 2], I32)
            nc.sync.dma_start(hidx_sb[:], hidx[:])
            tidx_sb = idx_pool.tile([P, 8 * G], I16)
            nc.sync.dma_start(tidx_sb[:], tidx[:])
            rvals_sb = idx_pool.tile([P, M], F32)
            nc.scalar.dma_start(rvals_sb[:], rvals[:])
            riota_sb = idx_pool.tile([P, 4], F32)
            nc.scalar.dma_start(riota_sb[:], riota[:])
            ident_sb = idx_pool.tile([P, P], BF16)
            nc.scalar.dma_start(ident_sb[:], ident[:])
            relb_sb = gat_pool.tile([P, 4, E], BF16, tag="relsb")
            for t in range(4):
                nc.sync.dma_start(relb_sb[:, t, :], relb[t * P : (t + 1) * P, :])

            # --- Pool-engine DMAs (queues rewritten post-schedule):
            #     heads indirect x2, then the 16 tail window gathers ---
            tbig = gat_pool.tile([P, G, 2, P], BF16, tag="tbig")
            for w in range(G):
                base = tail_bases[w]
                nc.gpsimd.dma_gather(
                    out_ap=tbig[:, w, :, :],
                    in_ap=entb[base : base + WIN, :],
                    idxs_ap=tidx_sb[:, 8 * w : 8 * w + 8],
                    num_idxs=P, num_idxs_reg=P, elem_size=E,
                    transpose=True, queue_num=0,
                )
            hrows = gat_pool.tile([P, 2, E], BF16, tag="hrows")
            for j in range(2):
                nc.gpsimd.indirect_dma_start(
                    out=hrows[:, j, :],
                    out_offset=None,
                    in_=entb[:],
                    in_offset=bass.IndirectOffsetOnAxis(
                        ap=hidx_sb[:, j : j + 1], axis=0
                    ),
                )

            # --- one-hot rel selection, fold into heads, transpose ---
            # OH[r_part, t, m] = (batch_r[m] == 128t + r_part)
            oh = gat_pool.tile([P, 4, M], BF16, tag="oh")
            for t in range(4):
                nc.vector.tensor_scalar(
                    oh[:, t, :], rvals_sb[:], riota_sb[:, t : t + 1], None,
                    op0=mybir.AluOpType.is_equal,
                )

            # PE warmups (HAM clock ramp) before the one-hot matmuls
            wpsum = psum_wm.tile([P, P], F32)
            for _ in range(NWARM_EARLY):
                nc.tensor.matmul(wpsum[:], lhsT=wmt[:], rhs=wmt[:], start=True, stop=True)

            # rel_selT[e_tile k] = sum_t relb[t][:, ek]^T @ OH[:, t, :]
            #   -> [128e, 256m], i.e. already transposed; copy to SBUF while
            #   the tail gathers drain (PE+DVE are otherwise idle here)
            rselT = gat_pool.tile([P, 2, M], BF16, tag="rselT")
            for k in range(2):
                psr = psum_r.tile([P, M], F32, tag="psr", name=f"psr{k}")
                for t in range(4):
                    nc.tensor.matmul(
                        psr[:],
                        lhsT=relb_sb[:, t, k * P : (k + 1) * P],
                        rhs=oh[:, t, :],
                        start=(t == 0),
                        stop=(t == 3),
                    )
                nc.vector.tensor_copy(rselT[:, k, :], psr[:])

            # bridge warmups while gathers land
            for _ in range(NWARM_BRIDGE):
                nc.tensor.matmul(wpsum[:], lhsT=wmt[:], rhs=wmt[:], start=True, stop=True)

            # transpose head rows as they land; hrT = headsT * rel_selT
            hrT = gat_pool.tile([P, 2, M], BF16, tag="hrT")
            pst_tiles = []
            for k in range(2):
                pst = psum_t.tile([P, M], BF16, tag="pst", name=f"pst{k}")
                for i in range(2):
                    nc.tensor.transpose(
                        pst[:, i * P : (i + 1) * P],
                        hrows[:, i, k * P : (k + 1) * P],
                        ident_sb[:],
                    )
                pst_tiles.append(pst)
            for k in range(2):
                nc.vector.tensor_mul(hrT[:, k, :], pst_tiles[k][:], rselT[:, k, :])

            # --- score matmuls + sigmoid + out, n-chunks of 512 cols ---
            n_chunks = (G + 3) // 4
            for c in range(n_chunks):
                w0 = 4 * c
                wn = min(4, G - w0)
                ncols = wn * P
                for i in range(M // P):
                    ps = psum_mm.tile([P, ncols], F32, tag="ps", name=f"ps{c}_{i}")
                    for k in range(2):
                        nc.tensor.matmul(
                            ps[:],
                            lhsT=hrT[:, k, i * P : (i + 1) * P],
                            rhs=tbig[:, w0 : w0 + wn, k, :],
                            start=(k == 0),
                            stop=(k == 1),
                        )
                    ob = out_pool.tile([P, ncols], BF16, tag="ob", name=f"ob{c}_{i}")
                    nc.scalar.activation(
                        ob[:], ps[:], mybir.ActivationFunctionType.Sigmoid
                    )
                    nc.sync.dma_start(
                        score[i * P : (i + 1) * P, w0 * P : w0 * P + ncols], ob[:]
                    )

    # Tile assigns DMASW completion sems round-robin over 8 lanes in
    # scheduled order, and SWDGE shadow-sem bookkeeping requires each sem to
    # be driven by exactly one queue. Rewrite each gather's queue to a pure
    # function of its lane; lanes used by the queue-0-only indirect DMAs
    # stay on queue 0, the rest spread evenly over queues 1..3.
    import re as _re

    pool_dmas = []
    for bb in nc.main_func.blocks:
        for inst in bb.instructions:
            if inst.engine != mybir.EngineType.Pool:
                continue
            si = inst.sync_info
            if not si or not si.on_update:
                continue
            m = _re.match(r"DMASW(\d+)_", si.on_update[0].ant_name or "")
            if not m:
                continue
            pool_dmas.append((inst, int(m.group(1))))
    indirect_lanes = {
        lane for inst, lane in pool_dmas if isinstance(inst, mybir.InstDMACopy)
    }
    qmap = {lane: 0 for lane in indirect_lanes}
    free_lanes = [ln for ln in range(8) if ln not in indirect_lanes]
    fill = ([1, 2, 3] if indirect_lanes else [0, 1, 2, 3]) * 8
    for i, ln in enumerate(free_lanes):
        qmap[ln] = fill[i]
    for inst, lane in pool_dmas:
        if isinstance(inst, mybir.InstDMAGatherAnt):
            inst.queue_num = qmap[lane]

    nc.compile()
    return nc


_NC_CACHE = {}


def _get_nc(G, tail_bases):
    key = (G, tuple(tail_bases))
    if key not in _NC_CACHE:
        _NC_CACHE[key] = build_nc(G, tail_bases)
    return _NC_CACHE[key]


def _wrap16(idx, reps=8):
    """Position i of a gather call reads idxs[i % 16, i // 16]; replicate to 128 rows."""
    n = idx.shape[0]
    w = idx.reshape(n // 16, 16).T
    return np.ascontiguousarray(np.tile(w, (reps, 1)))


def _plan_tail_buckets(bt_sorted):
    """Greedy exact-128 buckets of sorted tail indices; pad a bucket (repeating
    its first index) when 128 consecutive sorted values span > 32767 rows.
    Returns (bases, lo_idx [G*128] int16, keep [G*128] bool)."""
    n = bt_sorted.shape[0]
    bases, lo_all, keep_all = [], [], []
    pos = 0
    while pos < n:
        chunk = bt_sorted[pos : pos + P]
        span = int(chunk[-1]) - int(chunk[0])
        if span <= WIN - 1:
            take = len(chunk)
        else:
            take = int(np.searchsorted(chunk, chunk[0] + WIN, side="left"))
        vals = chunk[:take]
        pad = P - take
        if pad:
            vals = np.concatenate([vals, np.full(pad, vals[0], dtype=vals.dtype)])
        base = min(int(vals.min()), N_ENT - WIN)
        bases.append(base)
        lo_all.append((vals - base).astype(np.int16))
        keep_all.append(np.arange(P) < take)
        pos += take
    return bases, np.concatenate(lo_all), np.concatenate(keep_all)


def prepare(batch_h, batch_t, batch_r, ent_emb, rel_emb):
    bh = np.asarray(batch_h).astype(np.int64)
    bt = np.asarray(batch_t).astype(np.int64)
    br = np.asarray(batch_r).astype(np.int64)

    entb = np.asarray(ent_emb).astype(ml_dtypes.bfloat16)
    rel_np = np.asarray(rel_emb)
    rel_diag = rel_np[:, np.arange(E), np.arange(E)].astype(ml_dtypes.bfloat16)
    relb = np.zeros((512, E), dtype=ml_dtypes.bfloat16)
    relb[:N_REL] = rel_diag

    # tails: global sort -> window buckets (shared by all cores)
    t_order = np.argsort(bt, kind="stable")
    bases, t_lo, t_keep = _plan_tail_buckets(bt[t_order])
    G = len(bases)
    tidx = _wrap16(t_lo)  # [128, 8G]

    # heads: global sort -> per-core slices of 256
    h_order = np.argsort(bh, kind="stable")
    riota = np.ascontiguousarray(
        np.arange(P, dtype=np.float32)[:, None]
        + 128.0 * np.arange(4, dtype=np.float32)[None, :]
    )
    identity = np.eye(P, dtype=ml_dtypes.bfloat16)

    in_maps = []
    for c in range(CORES):
        rows = h_order[c * M : (c + 1) * M]  # original batch positions
        hvals = bh[rows].astype(np.int32)
        hidx = np.ascontiguousarray(hvals.reshape(2, P).T)  # [128, 2] col j = rows 128j+p
        rv = br[rows].astype(np.float32)
        rvals = np.ascontiguousarray(np.tile(rv[None, :], (P, 1)))
        in_maps.append(
            {
                "entb": entb, "relb": relb, "tidx": tidx, "hidx": hidx,
                "rvals": rvals, "riota": riota, "ident": identity,
            }
        )
    meta = {
        "G": G, "bases": tuple(int(b) for b in bases),
        "t_order": t_order, "t_keep": t_keep, "h_order": h_order,
    }
    return in_maps, meta


def run(batch_h, batch_t, batch_r, ent_emb, rel_emb, trace=False, tmpdir=None):
    from concourse.bass_utils import run_bass_kernel_spmd

    in_maps, meta = prepare(batch_h, batch_t, batch_r, ent_emb, rel_emb)
    nc = _get_nc(meta["G"], meta["bases"])
    kwargs = {}
    if trace:
        kwargs = {"trace": True, "tmpdir": tmpdir}
    res = run_bass_kernel_spmd(nc, in_maps, core_ids=list(range(CORES)), **kwargs)

    keep = meta["t_keep"]
    col_src = np.nonzero(keep)[0]  # device cols holding real sorted positions
    t_cols = meta["t_order"]  # sorted position -> original batch column
    full = np.empty((B, B), dtype=np.float32)
    for c in range(CORES):
        blk = np.asarray(res.results[c]["score"])  # [256, 128G] bf16
        rows = meta["h_order"][c * M : (c + 1) * M]
        full[np.ix_(rows, t_cols)] = blk[:, col_src].astype(np.float32)
    return full, res


def kernel(batch_h, batch_t, batch_r, ent_emb, rel_emb):
    score, _ = run(batch_h, batch_t, batch_r, ent_emb, rel_emb)
    return score
